# revision 28
# baseline (speedup 1.0000x reference)
"""Trainium2 Bass kernel for batched box-constrained QP projection (FISTA).

Per sample s (B=8192 total, data-parallel over 8 cores):
    min_x 0.5||x - x_raw||^2 + p*||relu(A x - b)||^2,  0 <= x <= 100
The objective is 1-strongly-convex with Lipschitz gradient
L = 1 + 2p*sigma_max(A)^2 (kappa ~ 9 for this ensemble), so accelerated
gradient with the CONSTANT strongly-convex momentum
beta = (sqrt(L)-1)/(sqrt(L)+1) converges linearly (~0.67/iter):
20 iterations reach the bf16 noise floor (~9e-4 rel err vs the
200-iteration t-sequence reference).

Per-core layout (1024 samples, 8 blocks of 128 = 2 halves of 64):
  - matvecs z=A y / w=A^T r run on the PE via per-sample "diagonal
    stationary" blocks in bf16 (4x fewer PE cycles/row than fp32):
    lhsT is an [K,32] block that is all zeros except column (p mod 32)
    holding the sample's vector; with tile_position=(0,32*(p//32)) the
    result lands in psum row p (fp32 accumulate).
  - all pointwise math runs batched fp32 on [64, N] tiles (DVE),
  - per iteration a PE transpose + one strided DVE scatter (fp32 psum ->
    bf16 cast) rebuilds the diagonal stationaries from updated y / r.
"""
import dataclasses
import math
from contextlib import ExitStack

import numpy as np
import ml_dtypes

import concourse.bass as bass
import concourse.tile as tile
from concourse import mybir
from concourse.bass import ds
from concourse.bass_utils import run_bass_kernel_spmd
from concourse.masks import make_identity

# problem constants (hardcoded per spec)
B_TOTAL = 8192
N_CORES = 8
B_CORE = B_TOTAL // N_CORES       # 1024
BLK = 128                          # samples per block
H = 64                             # samples per half
NBLK = B_CORE // BLK               # 8
N = 80                             # x dim
M = 85                             # constraint dim
P_SLACK = 1.0
ITERS = 16                         # FISTA iterations (multiple of UNROLL)
UNROLL = 16
PITERS = 2                         # power iterations
LSAFETY = 1.08                     # multiplier on L (covers short power iter)
F32 = mybir.dt.float32
BF16 = mybir.dt.bfloat16


def _diag_dest(region_ap, half):
    """Scatter destination: for local sample p (0..63) of `half`, block
    b = 64*half + p occupies cols [32b, 32b+32); the vector goes to column
    offset (p mod 32).  col = 2048*half + 1024*(p//32) + 33*(p%32)."""
    pstride, pcount = region_ap.ap[0]
    return dataclasses.replace(
        region_ap,
        offset=region_ap.offset + 2048 * half,
        ap=[[pstride, pcount], [1024, 2], [33, 32]],
    )


import contextlib as _ctxlib
_CRIT = True


def _mv_ctx(tc):
    return tc.tile_critical() if _CRIT else _ctxlib.nullcontext()


def _emit_matvec_all(nc, banks, diag_region, mov_buf, mov_cols, skip=True):
    """128 matmuls for both halves, col-groups 0..3 round-robin so each
    implicit LDWEIGHTS overlaps the other groups' in-flight MM."""
    for o in range(32):
        for c in range(4):
            blk_i = 32 * c + o
            out = banks[c // 2][32 * c:32 * c + 32, 0:mov_cols]
            lhsT = diag_region[:, 32 * blk_i:32 * blk_i + 32]
            rhs = mov_buf[:, mov_cols * blk_i:mov_cols * blk_i + mov_cols]
            nc.tensor.matmul(
                out, lhsT, rhs,
                start=(o == 0), stop=(o == 31),
                tile_position=(0, 32 * c), skip_group_check=skip,
            )


_INJECT_MODE = "mid"   # "mid" | "after" | "before"


def _emit_matvec_half(nc, bank, diag_region, mov_buf, mov_cols, half,
                      inject_at=None, inject_fn=None, skip=True):
    """64 matmuls for one half (col-groups 2h, 2h+1 alternating so each
    implicit LDWEIGHTS overlaps the other group's in-flight MM).  If
    inject_fn is given it is emitted after MM #inject_at — used to slot a
    PE transpose (+DVE scatter) mid-chunk so its consumers see it complete
    before the next chunk starts (software-pipelined rotation)."""
    if _INJECT_MODE == "before" and inject_fn is not None:
        inject_fn()
        inject_fn = None
    idx = 0
    for o in range(32):
        for cl in range(2):
            if inject_fn is not None and idx == inject_at and _INJECT_MODE == "mid":
                inject_fn()
                inject_fn = None
            c = 2 * half + cl
            blk_i = 32 * c + o
            nc.tensor.matmul(
                bank[32 * c:32 * c + 32, 0:mov_cols],
                diag_region[:, 32 * blk_i:32 * blk_i + 32],
                mov_buf[:, mov_cols * blk_i:mov_cols * blk_i + mov_cols],
                start=(o == 0), stop=(o == 31),
                tile_position=(0, 32 * c), skip_group_check=skip,
            )
            idx += 1
    if inject_fn is not None:   # mode "after" (or idx never reached)
        inject_fn()


def _split_multiwait_insts(nc):
    """walrus codegen allows only ONE sync-wait on compute/Drain instructions
    (setupSyncWait: 'Too many sync wait commands').  Tile can emit several.
    Peel all-but-one wait off onto same-engine single-wait NoOps placed just
    before the instruction (same engine + program order => identical
    semantics).  Barrier NoOps are left untouched."""
    cnt = 0
    for f in nc.m.functions:
        for b in f.blocks:
            il = list(b.instructions)
            out = []
            changed = False
            for ins in il:
                si = getattr(ins, "sync_info", None)
                if (
                    si is not None
                    and len(si.on_wait) > 1
                    and ins.opcode != "ISA"
                ):
                    waits = list(si.on_wait)
                    for j, w in enumerate(waits[:-1]):
                        nd = mybir.InstDrain(
                            name=f"{ins.name}-sw{j}", engine=ins.engine,
                            ins=[], outs=[],
                        )
                        nd.sync_info = mybir.SyncInfo(on_wait=[w], on_update=[])
                        out.append(nd)
                        cnt += 1
                    ins.sync_info = mybir.SyncInfo(
                        on_wait=[waits[-1]], on_update=list(si.on_update)
                    )
                    changed = True
                out.append(ins)
            if changed:
                b.instructions = out
    return cnt


def build_kernel(nc, split_waits=True, sim_unroll=False, nblk=NBLK):
    # A layouts padded with one dummy block so the steady-state prefetch of
    # block bi+1 stays in bounds on the last block
    x_raw_d = nc.dram_tensor("x_raw", [B_CORE, N], F32, kind="ExternalInput").ap()
    A_d = nc.dram_tensor("Ap", [NBLK + 1, M, BLK * N], BF16, kind="ExternalInput").ap()
    AT_d = nc.dram_tensor("ATp", [NBLK + 1, N, BLK * M], BF16, kind="ExternalInput").ap()
    b_d = nc.dram_tensor("b", [B_CORE, M], F32, kind="ExternalInput").ap()
    out_d = nc.dram_tensor("x_out", [B_CORE, N], F32, kind="ExternalOutput").ap()

    with tile.TileContext(nc) as tc, ExitStack() as ctx:
        consts = ctx.enter_context(tc.tile_pool(name="consts", bufs=1))
        abuf = ctx.enter_context(tc.tile_pool(name="abuf", bufs=1))
        state = ctx.enter_context(tc.tile_pool(name="state", bufs=1))
        ps = ctx.enter_context(tc.tile_pool(name="ps", bufs=1, space="PSUM"))

        ident = consts.tile([128, 128], F32)
        make_identity(nc, ident)

        # diagonal stationary regions (off-diagonal zeros persist forever)
        y_diag = consts.tile([N, 32 * BLK], BF16)
        r_diag = consts.tile([M, 32 * BLK], BF16)
        nc.vector.memset(y_diag[:], 0.0)
        nc.vector.memset(r_diag[:], 0.0)

        # per-block A buffers (sample-major along free dim); two sets so the
        # next block's A streams in during the current block's compute
        AT_bufs = [abuf.tile([N, BLK * M], BF16, name=f"AT_buf{i}")
                   for i in range(2)]            # [n, 85*b + m]
        A_bufs = [abuf.tile([M, BLK * N], BF16, name=f"A_buf{i}")
                  for i in range(2)]             # [m, 80*b + n]

        # per-half state tiles: halves of [128, x] parents so that every
        # SB operand of a half shares the same base partition (64*h)
        def half_tiles(name, cols):
            t = state.tile([BLK, cols], F32, name=name)
            return t, [t[H * hh:H * hh + H, :] for hh in range(2)]
        y_t, y_sb = half_tiles("y_t", N)
        v_t, v_sb = half_tiles("v_t", N)    # power-iteration vector
        xa_t, xa = half_tiles("xa_t", N)
        xb_t, xb = half_tiles("xb_t", N)
        xraw_t, xraw_sb = half_tiles("xraw_t", N)
        b_t, b_sb = half_tiles("b_t", M)
        r_t, r_sb = half_tiles("r_t", M)
        g_t, g_sb = half_tiles("g_t", N)
        u_t, u_sb = half_tiles("u_t", N)
        av_t, av_sb = half_tiles("av_t", M)
        # nrm2, rinv, rs, sig2, L, step, negstep, beta
        sc_t, sc_sb = half_tiles("sc_t", 8)

        # psum tiles (one bank each); half h occupies rows [64h, 64h+64)
        z_ps_t = [ps.tile([128, 512], F32, name=f"z{h}") for h in range(2)]
        w_ps_t = [ps.tile([128, 512], F32, name=f"w{h}") for h in range(2)]
        t1_ps_t = [ps.tile([128, 512], F32, name=f"t1{h}") for h in range(2)]
        t2_ps_t = [ps.tile([128, 512], F32, name=f"t2{h}") for h in range(2)]
        z_ps = [z_ps_t[hh][H * hh:H * hh + H, 0:M] for hh in range(2)]
        w_ps = [w_ps_t[hh][H * hh:H * hh + H, 0:N] for hh in range(2)]

        def scatter(dst_region, src_T, half):
            # src_T: psum [dim, 64]; dst: diag blocks of `half` (bf16 cast)
            nc.vector.tensor_copy(
                _diag_dest(dst_region, half),
                src_T.rearrange("x (c o) -> x c o", o=32),
            )

        def transpose_scatter(vec_sb, dst_region, t_tile, half, dim):
            tp = t_tile[0:dim, 0:H]
            idh = ident[H * half:H * half + H, H * half:H * half + H]
            nc.tensor.transpose(tp, vec_sb[:, 0:dim], idh)
            scatter(dst_region, tp, half)

        cur = {"A": A_bufs[0], "AT": AT_bufs[0]}

        def emit_pow_iter(_pi, last=False):
            """One rotated-pipeline power iteration.  Non-last iterations
            renormalize v into y_sb; the last only needs nrm2 = ||A^T A v||^2
            (norm-growth estimate: sigma_max^2 ~= sqrt(nrm2))."""
            # z half0; inject prev iteration's v1 transpose (iteration 0:
            # re-scatters the all-ones v0, idempotent)
            _emit_matvec_half(
                nc, z_ps_t[0][:], y_diag, cur["AT"], M, 0,
                inject_at=26,
                inject_fn=lambda: transpose_scatter(
                    v_sb[1], y_diag, t1_ps_t[1], 1, N))
            nc.vector.tensor_copy(av_sb[0][:], z_ps[0])
            _emit_matvec_half(
                nc, z_ps_t[1][:], y_diag, cur["AT"], M, 1,
                inject_at=12,
                inject_fn=lambda: transpose_scatter(
                    av_sb[0], r_diag, t2_ps_t[0], 0, M))
            nc.vector.tensor_copy(av_sb[1][:], z_ps[1])
            _emit_matvec_half(
                nc, w_ps_t[0][:], r_diag, cur["A"], N, 0,
                inject_at=12,
                inject_fn=lambda: transpose_scatter(
                    av_sb[1], r_diag, t2_ps_t[1], 1, M))

            def norm(h):
                nrm2 = sc_sb[h][:, 0:1]
                rinv = sc_sb[h][:, 1:2]
                rs = sc_sb[h][:, 2:3]
                nc.scalar.activation(g_sb[h][:], w_ps[h],
                                     mybir.ActivationFunctionType.Square)
                nc.vector.reduce_sum(nrm2, g_sb[h][:], axis=mybir.AxisListType.X)
                if not last:
                    nc.vector.reciprocal(rinv, nrm2)
                    nc.scalar.sqrt(rs, rinv)
                    nc.vector.tensor_scalar_mul(v_sb[h][:], w_ps[h], rs)

            norm(0)
            if last:
                # ride the FISTA-entry y0 (= x0) scatter in this chunk: the
                # first FISTA z0 then starts with no PE gap
                _emit_matvec_half(
                    nc, w_ps_t[1][:], r_diag, cur["A"], N, 1,
                    inject_at=34,
                    inject_fn=lambda: transpose_scatter(
                        y_sb[0], y_diag, t1_ps_t[0], 0, N))
            else:
                _emit_matvec_half(
                    nc, w_ps_t[1][:], r_diag, cur["A"], N, 1,
                    inject_at=34,
                    inject_fn=lambda: transpose_scatter(
                        v_sb[0], y_diag, t1_ps_t[0], 0, N))
            norm(1)

        def emit_a_dma(bi, bufi):
            nc.sync.dma_start(AT_bufs[bufi][:],
                              AT_d[ds(bi, 1), :, :].rearrange("o n x -> (o n) x"))
            nc.sync.dma_start(A_bufs[bufi][:],
                              A_d[ds(bi, 1), :, :].rearrange("o m x -> (o m) x"))

        def emit_block(bj, pair, bufi):
            # block index = 2*bj + pair; keep loop-var arithmetic in the
            # var*const + const form the symbolic AP lowering supports
            cur["A"], cur["AT"] = A_bufs[bufi], AT_bufs[bufi]
            # prefetch next block's A into the other buffer (its previous
            # reader, block bi-1, is already done)
            emit_a_dma(bj * 2 + (pair + 1), 1 - bufi)
            for h in range(2):
                off = pair * BLK + H * h
                nc.sync.dma_start(xraw_sb[h][:],
                                  x_raw_d[ds(bj * (2 * BLK) + off, H), :])
                nc.sync.dma_start(b_sb[h][:], b_d[ds(bj * (2 * BLK) + off, H), :])

            # x0 = clip(x_raw) and y0 = x0: independent of the power
            # iteration, emit early so they hide under the matvecs
            for h in range(2):
                nc.vector.tensor_scalar(
                    xb[h], xraw_sb[h][:], 0.0, 100.0,
                    op0=mybir.AluOpType.max, op1=mybir.AluOpType.min,
                )
                nc.vector.tensor_copy(y_sb[h][:], xb[h])

            # ---- power iteration: v <- normalize(A^T A v), v0 = const ----
            # v lives in its own tile so this can overlap the previous
            # block's FISTA tail (y_sb is still live there)
            for h in range(2):
                nc.vector.memset(v_sb[h][:], 1.0)
            transpose_scatter(v_sb[0], y_diag, t1_ps_t[0], 0, N)
            for p in range(PITERS):
                emit_pow_iter(p, last=(p == PITERS - 1))

            # ---- sigma^2 ~= sqrt(||A^T A v||^2);  L = LSAFETY*(1+2p*s^2) ----
            for h in range(2):
                nrm2 = sc_sb[h][:, 0:1]
                num = sc_sb[h][:, 1:2]
                den = sc_sb[h][:, 2:3]
                sig2 = sc_sb[h][:, 3:4]
                L = sc_sb[h][:, 4:5]
                step = sc_sb[h][:, 5:6]
                negstep = sc_sb[h][:, 6:7]
                beta = sc_sb[h][:, 7:8]
                nc.scalar.sqrt(sig2, nrm2)
                nc.vector.tensor_scalar(
                    L, sig2, 2.0 * P_SLACK * LSAFETY, LSAFETY,
                    op0=mybir.AluOpType.mult, op1=mybir.AluOpType.add,
                )
                nc.vector.reciprocal(step, L)
                nc.vector.tensor_scalar_mul(negstep, step, -1.0)
                # beta = (1-rs)/(1+rs) with rs = sqrt(step) = 1/sqrt(L)
                rs2 = sc_sb[h][:, 0:1]  # nrm2 dead now
                nc.scalar.sqrt(rs2, step)
                nc.vector.tensor_scalar(
                    num, rs2, -1.0, 1.0,
                    op0=mybir.AluOpType.mult, op1=mybir.AluOpType.add,
                )
                nc.vector.tensor_scalar(
                    den, rs2, 1.0, 1.0,
                    op0=mybir.AluOpType.mult, op1=mybir.AluOpType.add,
                )
                nc.vector.reciprocal(den, den)
                nc.vector.tensor_mul(beta, num, den)

            # ---- FISTA iterations (constant strongly-convex momentum) ----
            # Rotated software pipeline (uniform body): each iteration's y
            # transposes are injected into the NEXT PE chunk so every
            # transpose+scatter lands mid-chunk, with slack before its
            # consumer chunk.  Iteration k=0 re-runs k=-1's y1 transpose,
            # which is idempotent (sigma section scattered the same y).
            def emit_pointwise(h, x_old, x_new):
                negstep = sc_sb[h][:, 6:7]
                beta = sc_sb[h][:, 7:8]
                # g = y - x_raw ; u = 2p*w + g
                nc.vector.tensor_sub(g_sb[h][:], y_sb[h][:], xraw_sb[h][:])
                nc.vector.scalar_tensor_tensor(
                    u_sb[h][:], w_ps[h], 2.0 * P_SLACK, g_sb[h][:],
                    op0=mybir.AluOpType.mult, op1=mybir.AluOpType.add,
                )
                # x_new = clip(y - step*u)
                nc.vector.scalar_tensor_tensor(
                    x_new, u_sb[h][:], negstep, y_sb[h][:],
                    op0=mybir.AluOpType.mult, op1=mybir.AluOpType.add,
                )
                nc.vector.tensor_scalar(
                    x_new, x_new, 0.0, 100.0,
                    op0=mybir.AluOpType.max, op1=mybir.AluOpType.min,
                )
                # y = x_new + beta*(x_new - x_old)
                nc.vector.tensor_sub(g_sb[h][:], x_new, x_old)
                nc.vector.scalar_tensor_tensor(
                    y_sb[h][:], g_sb[h][:], beta,
                    x_new,
                    op0=mybir.AluOpType.mult, op1=mybir.AluOpType.add,
                )

            def emit_fista_iter(k):
                x_old = [xb, xa][k % 2]
                x_new = [xa, xb][k % 2]
                # z half0; inject prev iter's y1 transpose
                _emit_matvec_half(
                    nc, z_ps_t[0][:], y_diag, cur["AT"], M, 0,
                    inject_at=26,
                    inject_fn=lambda: transpose_scatter(
                        y_sb[1], y_diag, t1_ps_t[1], 1, N))
                nc.vector.tensor_sub(av_sb[0][:], z_ps[0], b_sb[0][:])
                nc.scalar.activation(r_sb[0][:], av_sb[0][:],
                                     mybir.ActivationFunctionType.Relu)
                # z half1; inject r0 transpose
                _emit_matvec_half(
                    nc, z_ps_t[1][:], y_diag, cur["AT"], M, 1,
                    inject_at=12,
                    inject_fn=lambda: transpose_scatter(
                        r_sb[0], r_diag, t2_ps_t[0], 0, M))
                nc.vector.tensor_sub(av_sb[1][:], z_ps[1], b_sb[1][:])
                nc.scalar.activation(r_sb[1][:], av_sb[1][:],
                                     mybir.ActivationFunctionType.Relu)
                # w half0; inject r1 transpose
                _emit_matvec_half(
                    nc, w_ps_t[0][:], r_diag, cur["A"], N, 0,
                    inject_at=12,
                    inject_fn=lambda: transpose_scatter(
                        r_sb[1], r_diag, t2_ps_t[1], 1, M))
                emit_pointwise(0, x_old[0], x_new[0])
                # w half1; inject y0 transpose (needs pw0)
                _emit_matvec_half(
                    nc, w_ps_t[1][:], r_diag, cur["A"], N, 1,
                    inject_at=34,
                    inject_fn=lambda: transpose_scatter(
                        y_sb[0], y_diag, t1_ps_t[0], 0, N))
                emit_pointwise(1, x_old[1], x_new[1])

            def emit_fista_group(t0):
                for k in range(UNROLL):
                    emit_fista_iter(k)

            if sim_unroll:
                for t0 in range(0, ITERS, UNROLL):
                    emit_fista_group(t0)
            else:
                with tc.For_i(0, ITERS, UNROLL, name="fista",
                              hint_engines=(mybir.EngineType.PE,)) as t0:
                    emit_fista_group(t0)

            # final x lives in the tile written by iteration ITERS-1 (k=3 -> xb)
            nc.sync.dma_start(out_d[ds(bj * (2 * BLK) + pair * BLK, BLK), :], xb_t[:])

        # block 0's A arrives before the loop; inside, each block prefetches
        # the next block's A into the buffer its predecessor just vacated
        emit_a_dma(0, 0)
        if sim_unroll:
            for bj in range(nblk // 2):
                emit_block(bj, 0, 0)
                emit_block(bj, 1, 1)
        else:
            with tc.For_i(0, NBLK // 2, 1, name="blk") as bj:
                emit_block(bj, 0, 0)
                emit_block(bj, 1, 1)

    if split_waits:
        _split_multiwait_insts(nc)
    return nc


_CACHED = {}


def _get_nc():
    if "nc" not in _CACHED:
        nc = bass.Bass("TRN2", target_bir_lowering=False, debug=False)
        build_kernel(nc)
        nc.finalize()
        _CACHED["nc"] = nc
    return _CACHED["nc"]


def _concat_in_maps(x_raw, A, b):
    per_core = []
    zA = np.zeros((1, M, BLK * N), ml_dtypes.bfloat16)
    zAT = np.zeros((1, N, BLK * M), ml_dtypes.bfloat16)
    for c in range(N_CORES):
        sl = slice(c * B_CORE, (c + 1) * B_CORE)
        Ac = A[sl].reshape(NBLK, BLK, M, N).astype(ml_dtypes.bfloat16)
        Ap = np.ascontiguousarray(Ac.transpose(0, 2, 1, 3)).reshape(NBLK, M, BLK * N)
        ATp = np.ascontiguousarray(Ac.transpose(0, 3, 1, 2)).reshape(NBLK, N, BLK * M)
        per_core.append({
            "x_raw": x_raw[sl],
            "Ap": np.concatenate([Ap, zA], axis=0),
            "ATp": np.concatenate([ATp, zAT], axis=0),
            "b": b[sl],
        })
    return per_core


def _build_trivial_nc():
    """Minimal DMA-roundtrip kernel used to calibrate the axon dispatch
    floor with the exact same timing path as the real kernel."""
    import concourse.tile as tile_mod
    nc = bass.Bass("TRN2", target_bir_lowering=False, debug=False)
    inp = nc.dram_tensor("tin", [128, 128], F32, kind="ExternalInput").ap()
    out = nc.dram_tensor("tout", [128, 128], F32, kind="ExternalOutput").ap()
    with tile_mod.TileContext(nc) as tc:
        with tc.tile_pool(name="p", bufs=1) as pool:
            t = pool.tile([128, 128], F32)
            nc.sync.dma_start(t[:], inp)
            nc.sync.dma_start(out, t[:])
    _split_multiwait_insts(nc)
    nc.finalize()
    return nc


def timed_runs_trivial(n=5):
    if "triv" not in _CACHED:
        _CACHED["triv"] = _build_trivial_nc()
    per_core = [{"tin": np.zeros((128, 128), np.float32)} for _ in range(N_CORES)]
    return _timed_exec(_CACHED["triv"], per_core, n)


def timed_runs(inputs, n=5):
    """Warm, device-resident-input executions; returns per-call wall ns."""
    x_raw = np.ascontiguousarray(inputs["x_raw"], np.float32)
    A = np.ascontiguousarray(inputs["A"], np.float32)
    b = np.ascontiguousarray(inputs["b"], np.float32)
    per_core = _concat_in_maps(x_raw, A, b)
    return _timed_exec(_get_nc(), per_core, n)


def timed_interleaved(inputs, n=24):
    """Alternate real-kernel and trivial-kernel timed calls so both sample
    the same axon-tunnel dispatch regime; returns (real_ns, trivial_ns)."""
    import time
    import jax

    x_raw = np.ascontiguousarray(inputs["x_raw"], np.float32)
    A = np.ascontiguousarray(inputs["A"], np.float32)
    b = np.ascontiguousarray(inputs["b"], np.float32)
    per_core = _concat_in_maps(x_raw, A, b)
    fn_r, args_r = _timed_prep(_get_nc(), per_core)
    if "triv" not in _CACHED:
        _CACHED["triv"] = _build_trivial_nc()
    triv_core = [{"tin": np.zeros((128, 128), np.float32)}
                 for _ in range(N_CORES)]
    fn_t, args_t = _timed_prep(_CACHED["triv"], triv_core)
    # warm both
    jax.block_until_ready(fn_r(*args_r))
    jax.block_until_ready(fn_t(*args_t))
    real, triv = [], []
    for _ in range(n):
        t0 = time.perf_counter()
        jax.block_until_ready(fn_t(*args_t))
        t1 = time.perf_counter()
        jax.block_until_ready(fn_r(*args_r))
        t2 = time.perf_counter()
        triv.append((t1 - t0) * 1e9)
        real.append((t2 - t1) * 1e9)
    return real, triv


def _timed_exec(nc, per_core, n):
    import time
    import jax

    fn, args = _timed_prep(nc, per_core)
    out = fn(*args)
    jax.block_until_ready(out)  # compile + warmup
    times = []
    for _ in range(n):
        t0 = time.perf_counter()
        out = fn(*args)
        jax.block_until_ready(out)
        times.append((time.perf_counter() - t0) * 1e9)
    return times


def _timed_prep(nc, per_core):
    import jax
    from jax.sharding import Mesh, PartitionSpec, NamedSharding
    from jax.experimental.shard_map import shard_map
    from concourse import bass2jax

    bass2jax.install_neuronx_cc_hook()

    in_names, out_names, out_avals = [], [], []
    for alloc in nc.m.functions[0].allocations:
        if not isinstance(alloc, mybir.MemoryLocationSet):
            continue
        name = alloc.memorylocations[0].name
        if alloc.kind == "ExternalInput":
            in_names.append(name)
        elif alloc.kind == "ExternalOutput":
            out_names.append(name)
            out_avals.append(jax.core.ShapedArray(
                tuple(alloc.tensor_shape), mybir.dt.np(alloc.dtype)))
    pid_name = nc.partition_id_tensor.name if nc.partition_id_tensor else None
    if pid_name is not None and pid_name in in_names:
        in_names.remove(pid_name)

    all_names = in_names + out_names
    if pid_name is not None:
        all_names = all_names + [pid_name]

    def _body(*args):
        operands = list(args)
        if pid_name is not None:
            operands.append(bass2jax.partition_id_tensor())
        outs = bass2jax._bass_exec_p.bind(
            *operands,
            out_avals=tuple(out_avals),
            in_names=tuple(all_names),
            out_names=tuple(out_names),
            lowering_input_output_aliases=(),
            sim_require_finite=True,
            sim_require_nnan=True,
            nc=nc,
        )
        return tuple(outs)

    devices = jax.devices()[:N_CORES]
    mesh = Mesh(np.asarray(devices), ("core",))
    nin = len(in_names) + len(out_names)
    fn = jax.jit(
        shard_map(_body, mesh=mesh, in_specs=(PartitionSpec("core"),) * nin,
                  out_specs=(PartitionSpec("core"),) * len(out_names),
                  check_rep=False),
        keep_unused=True,
    )
    sh = NamedSharding(mesh, PartitionSpec("core"))
    concat = [np.concatenate([pc[nm] for pc in per_core], axis=0) for nm in in_names]
    zeros = [np.zeros((N_CORES * av.shape[0], *av.shape[1:]), av.dtype)
             for av in out_avals]
    args = [jax.device_put(v, sh) for v in concat + zeros]
    return fn, args


def kernel(x_raw, A, b, lower, upper):
    x_raw = np.ascontiguousarray(x_raw, np.float32)
    A = np.ascontiguousarray(A, np.float32)
    b = np.ascontiguousarray(b, np.float32)

    nc = _get_nc()
    in_maps = _concat_in_maps(x_raw, A, b)
    res = run_bass_kernel_spmd(nc, in_maps, core_ids=list(range(N_CORES)))
    out = np.concatenate([res.results[c]["x_out"] for c in range(N_CORES)], axis=0)
    return out.astype(np.float32)


# revision 29
# speedup vs baseline: 2.2517x; 2.2517x over previous
"""Trainium2 Bass kernel for batched box-constrained QP projection (FISTA).

Per sample s (B=8192 total, data-parallel over 8 cores):
    min_x 0.5||x - x_raw||^2 + p*||relu(A x - b)||^2,  0 <= x <= 100
The objective is 1-strongly-convex with Lipschitz gradient
L = 1 + 2p*sigma_max(A)^2 (kappa ~ 9 for this ensemble), so accelerated
gradient with the CONSTANT strongly-convex momentum
beta = (sqrt(L)-1)/(sqrt(L)+1) converges linearly (~0.67/iter):
20 iterations reach the bf16 noise floor (~9e-4 rel err vs the
200-iteration t-sequence reference).

Per-core layout (1024 samples, 8 blocks of 128 = 2 halves of 64):
  - matvecs z=A y / w=A^T r run on the PE via per-sample "diagonal
    stationary" blocks in bf16 (4x fewer PE cycles/row than fp32):
    lhsT is an [K,32] block that is all zeros except column (p mod 32)
    holding the sample's vector; with tile_position=(0,32*(p//32)) the
    result lands in psum row p (fp32 accumulate).
  - all pointwise math runs batched fp32 on [64, N] tiles (DVE),
  - per iteration a PE transpose + one strided DVE scatter (fp32 psum ->
    bf16 cast) rebuilds the diagonal stationaries from updated y / r.
"""
import dataclasses
import math
from contextlib import ExitStack

import numpy as np
import ml_dtypes

import concourse.bass as bass
import concourse.tile as tile
from concourse import mybir
from concourse.bass import ds
from concourse.bass_utils import run_bass_kernel_spmd
from concourse.masks import make_identity

# problem constants (hardcoded per spec)
B_TOTAL = 8192
N_CORES = 8
B_CORE = B_TOTAL // N_CORES       # 1024
BLK = 128                          # samples per block
H = 64                             # samples per half
NBLK = B_CORE // BLK               # 8
N = 80                             # x dim
M = 85                             # constraint dim
P_SLACK = 1.0
ITERS = 16                         # FISTA iterations (multiple of UNROLL)
UNROLL = 4
PITERS = 2                         # power iterations
LSAFETY = 1.08                     # multiplier on L (covers short power iter)
F32 = mybir.dt.float32
BF16 = mybir.dt.bfloat16


def _diag_dest(region_ap, half):
    """Scatter destination: for local sample p (0..63) of `half`, block
    b = 64*half + p occupies cols [32b, 32b+32); the vector goes to column
    offset (p mod 32).  col = 2048*half + 1024*(p//32) + 33*(p%32)."""
    pstride, pcount = region_ap.ap[0]
    return dataclasses.replace(
        region_ap,
        offset=region_ap.offset + 2048 * half,
        ap=[[pstride, pcount], [1024, 2], [33, 32]],
    )


import contextlib as _ctxlib
_CRIT = True


def _mv_ctx(tc):
    return tc.tile_critical() if _CRIT else _ctxlib.nullcontext()


def _emit_matvec_all(nc, banks, diag_region, mov_buf, mov_cols, skip=True):
    """128 matmuls for both halves, col-groups 0..3 round-robin so each
    implicit LDWEIGHTS overlaps the other groups' in-flight MM."""
    for o in range(32):
        for c in range(4):
            blk_i = 32 * c + o
            out = banks[c // 2][32 * c:32 * c + 32, 0:mov_cols]
            lhsT = diag_region[:, 32 * blk_i:32 * blk_i + 32]
            rhs = mov_buf[:, mov_cols * blk_i:mov_cols * blk_i + mov_cols]
            nc.tensor.matmul(
                out, lhsT, rhs,
                start=(o == 0), stop=(o == 31),
                tile_position=(0, 32 * c), skip_group_check=skip,
            )


_INJECT_MODE = "mid"   # "mid" | "after" | "before"


def _emit_matvec_half(nc, bank, diag_region, mov_buf, mov_cols, half,
                      inject_at=None, inject_fn=None, skip=True):
    """64 matmuls for one half (col-groups 2h, 2h+1 alternating so each
    implicit LDWEIGHTS overlaps the other group's in-flight MM).  If
    inject_fn is given it is emitted after MM #inject_at — used to slot a
    PE transpose (+DVE scatter) mid-chunk so its consumers see it complete
    before the next chunk starts (software-pipelined rotation)."""
    if _INJECT_MODE == "before" and inject_fn is not None:
        inject_fn()
        inject_fn = None
    idx = 0
    for o in range(32):
        for cl in range(2):
            if inject_fn is not None and idx == inject_at and _INJECT_MODE == "mid":
                inject_fn()
                inject_fn = None
            c = 2 * half + cl
            blk_i = 32 * c + o
            nc.tensor.matmul(
                bank[32 * c:32 * c + 32, 0:mov_cols],
                diag_region[:, 32 * blk_i:32 * blk_i + 32],
                mov_buf[:, mov_cols * blk_i:mov_cols * blk_i + mov_cols],
                start=(o == 0), stop=(o == 31),
                tile_position=(0, 32 * c), skip_group_check=skip,
            )
            idx += 1
    if inject_fn is not None:   # mode "after" (or idx never reached)
        inject_fn()


def _split_multiwait_insts(nc):
    """walrus codegen allows only ONE sync-wait on compute/Drain instructions
    (setupSyncWait: 'Too many sync wait commands').  Tile can emit several.
    Peel all-but-one wait off onto same-engine single-wait NoOps placed just
    before the instruction (same engine + program order => identical
    semantics).  Barrier NoOps are left untouched."""
    cnt = 0
    for f in nc.m.functions:
        for b in f.blocks:
            il = list(b.instructions)
            out = []
            changed = False
            for ins in il:
                si = getattr(ins, "sync_info", None)
                if (
                    si is not None
                    and len(si.on_wait) > 1
                    and ins.opcode != "ISA"
                ):
                    waits = list(si.on_wait)
                    for j, w in enumerate(waits[:-1]):
                        nd = mybir.InstDrain(
                            name=f"{ins.name}-sw{j}", engine=ins.engine,
                            ins=[], outs=[],
                        )
                        nd.sync_info = mybir.SyncInfo(on_wait=[w], on_update=[])
                        out.append(nd)
                        cnt += 1
                    ins.sync_info = mybir.SyncInfo(
                        on_wait=[waits[-1]], on_update=list(si.on_update)
                    )
                    changed = True
                out.append(ins)
            if changed:
                b.instructions = out
    return cnt


def build_kernel(nc, split_waits=True, sim_unroll=False, nblk=NBLK):
    # A layouts padded with one dummy block so the steady-state prefetch of
    # block bi+1 stays in bounds on the last block
    x_raw_d = nc.dram_tensor("x_raw", [B_CORE, N], F32, kind="ExternalInput").ap()
    A_d = nc.dram_tensor("Ap", [NBLK + 1, M, BLK * N], BF16, kind="ExternalInput").ap()
    AT_d = nc.dram_tensor("ATp", [NBLK + 1, N, BLK * M], BF16, kind="ExternalInput").ap()
    b_d = nc.dram_tensor("b", [B_CORE, M], F32, kind="ExternalInput").ap()
    out_d = nc.dram_tensor("x_out", [B_CORE, N], F32, kind="ExternalOutput").ap()

    with tile.TileContext(nc) as tc, ExitStack() as ctx:
        consts = ctx.enter_context(tc.tile_pool(name="consts", bufs=1))
        abuf = ctx.enter_context(tc.tile_pool(name="abuf", bufs=1))
        state = ctx.enter_context(tc.tile_pool(name="state", bufs=1))
        ps = ctx.enter_context(tc.tile_pool(name="ps", bufs=1, space="PSUM"))

        ident = consts.tile([128, 128], F32)
        make_identity(nc, ident)

        # diagonal stationary regions (off-diagonal zeros persist forever)
        y_diag = consts.tile([N, 32 * BLK], BF16)
        r_diag = consts.tile([M, 32 * BLK], BF16)
        nc.vector.memset(y_diag[:], 0.0)
        nc.vector.memset(r_diag[:], 0.0)

        # per-block A buffers (sample-major along free dim); two sets so the
        # next block's A streams in during the current block's compute
        AT_bufs = [abuf.tile([N, BLK * M], BF16, name=f"AT_buf{i}")
                   for i in range(2)]            # [n, 85*b + m]
        A_bufs = [abuf.tile([M, BLK * N], BF16, name=f"A_buf{i}")
                  for i in range(2)]             # [m, 80*b + n]

        # per-half state tiles: halves of [128, x] parents so that every
        # SB operand of a half shares the same base partition (64*h)
        def half_tiles(name, cols):
            t = state.tile([BLK, cols], F32, name=name)
            return t, [t[H * hh:H * hh + H, :] for hh in range(2)]
        y_t, y_sb = half_tiles("y_t", N)
        v_t, v_sb = half_tiles("v_t", N)    # power-iteration vector
        xa_t, xa = half_tiles("xa_t", N)
        xb_t, xb = half_tiles("xb_t", N)
        xraw_t, xraw_sb = half_tiles("xraw_t", N)
        b_t, b_sb = half_tiles("b_t", M)
        r_t, r_sb = half_tiles("r_t", M)
        g_t, g_sb = half_tiles("g_t", N)
        u_t, u_sb = half_tiles("u_t", N)
        av_t, av_sb = half_tiles("av_t", M)
        # nrm2, rinv, rs, sig2, L, step, negstep, beta
        sc_t, sc_sb = half_tiles("sc_t", 8)

        # psum tiles (one bank each); half h occupies rows [64h, 64h+64)
        z_ps_t = [ps.tile([128, 512], F32, name=f"z{h}") for h in range(2)]
        w_ps_t = [ps.tile([128, 512], F32, name=f"w{h}") for h in range(2)]
        t1_ps_t = [ps.tile([128, 512], F32, name=f"t1{h}") for h in range(2)]
        t2_ps_t = [ps.tile([128, 512], F32, name=f"t2{h}") for h in range(2)]
        z_ps = [z_ps_t[hh][H * hh:H * hh + H, 0:M] for hh in range(2)]
        w_ps = [w_ps_t[hh][H * hh:H * hh + H, 0:N] for hh in range(2)]

        def scatter(dst_region, src_T, half):
            # src_T: psum [dim, 64]; dst: diag blocks of `half` (bf16 cast)
            nc.vector.tensor_copy(
                _diag_dest(dst_region, half),
                src_T.rearrange("x (c o) -> x c o", o=32),
            )

        def transpose_scatter(vec_sb, dst_region, t_tile, half, dim):
            tp = t_tile[0:dim, 0:H]
            idh = ident[H * half:H * half + H, H * half:H * half + H]
            nc.tensor.transpose(tp, vec_sb[:, 0:dim], idh)
            scatter(dst_region, tp, half)

        cur = {"A": A_bufs[0], "AT": AT_bufs[0]}

        def emit_pow_iter(_pi, last=False):
            """One rotated-pipeline power iteration.  Non-last iterations
            renormalize v into y_sb; the last only needs nrm2 = ||A^T A v||^2
            (norm-growth estimate: sigma_max^2 ~= sqrt(nrm2))."""
            # z half0; inject prev iteration's v1 transpose (iteration 0:
            # re-scatters the all-ones v0, idempotent)
            _emit_matvec_half(
                nc, z_ps_t[0][:], y_diag, cur["AT"], M, 0,
                inject_at=26,
                inject_fn=lambda: transpose_scatter(
                    v_sb[1], y_diag, t1_ps_t[1], 1, N))
            nc.vector.tensor_copy(av_sb[0][:], z_ps[0])
            _emit_matvec_half(
                nc, z_ps_t[1][:], y_diag, cur["AT"], M, 1,
                inject_at=12,
                inject_fn=lambda: transpose_scatter(
                    av_sb[0], r_diag, t2_ps_t[0], 0, M))
            nc.vector.tensor_copy(av_sb[1][:], z_ps[1])
            _emit_matvec_half(
                nc, w_ps_t[0][:], r_diag, cur["A"], N, 0,
                inject_at=12,
                inject_fn=lambda: transpose_scatter(
                    av_sb[1], r_diag, t2_ps_t[1], 1, M))

            def norm(h):
                nrm2 = sc_sb[h][:, 0:1]
                rinv = sc_sb[h][:, 1:2]
                rs = sc_sb[h][:, 2:3]
                nc.scalar.activation(g_sb[h][:], w_ps[h],
                                     mybir.ActivationFunctionType.Square)
                nc.vector.reduce_sum(nrm2, g_sb[h][:], axis=mybir.AxisListType.X)
                if not last:
                    nc.vector.reciprocal(rinv, nrm2)
                    nc.scalar.sqrt(rs, rinv)
                    nc.vector.tensor_scalar_mul(v_sb[h][:], w_ps[h], rs)

            norm(0)
            if last:
                # ride the FISTA-entry y0 (= x0) scatter in this chunk: the
                # first FISTA z0 then starts with no PE gap
                _emit_matvec_half(
                    nc, w_ps_t[1][:], r_diag, cur["A"], N, 1,
                    inject_at=34,
                    inject_fn=lambda: transpose_scatter(
                        y_sb[0], y_diag, t1_ps_t[0], 0, N))
            else:
                _emit_matvec_half(
                    nc, w_ps_t[1][:], r_diag, cur["A"], N, 1,
                    inject_at=34,
                    inject_fn=lambda: transpose_scatter(
                        v_sb[0], y_diag, t1_ps_t[0], 0, N))
            norm(1)

        def emit_a_dma(bi, bufi):
            nc.sync.dma_start(AT_bufs[bufi][:],
                              AT_d[ds(bi, 1), :, :].rearrange("o n x -> (o n) x"))
            nc.sync.dma_start(A_bufs[bufi][:],
                              A_d[ds(bi, 1), :, :].rearrange("o m x -> (o m) x"))

        def emit_block(bj, pair, bufi):
            # block index = 2*bj + pair; keep loop-var arithmetic in the
            # var*const + const form the symbolic AP lowering supports
            cur["A"], cur["AT"] = A_bufs[bufi], AT_bufs[bufi]
            # prefetch next block's A into the other buffer (its previous
            # reader, block bi-1, is already done)
            emit_a_dma(bj * 2 + (pair + 1), 1 - bufi)
            for h in range(2):
                off = pair * BLK + H * h
                nc.sync.dma_start(xraw_sb[h][:],
                                  x_raw_d[ds(bj * (2 * BLK) + off, H), :])
                nc.sync.dma_start(b_sb[h][:], b_d[ds(bj * (2 * BLK) + off, H), :])

            # x0 = clip(x_raw) and y0 = x0: independent of the power
            # iteration, emit early so they hide under the matvecs
            for h in range(2):
                nc.vector.tensor_scalar(
                    xb[h], xraw_sb[h][:], 0.0, 100.0,
                    op0=mybir.AluOpType.max, op1=mybir.AluOpType.min,
                )
                nc.vector.tensor_copy(y_sb[h][:], xb[h])

            # ---- power iteration: v <- normalize(A^T A v), v0 = const ----
            # v lives in its own tile so this can overlap the previous
            # block's FISTA tail (y_sb is still live there)
            for h in range(2):
                nc.vector.memset(v_sb[h][:], 1.0)
            transpose_scatter(v_sb[0], y_diag, t1_ps_t[0], 0, N)
            for p in range(PITERS):
                emit_pow_iter(p, last=(p == PITERS - 1))

            # ---- sigma^2 ~= sqrt(||A^T A v||^2);  L = LSAFETY*(1+2p*s^2) ----
            for h in range(2):
                nrm2 = sc_sb[h][:, 0:1]
                num = sc_sb[h][:, 1:2]
                den = sc_sb[h][:, 2:3]
                sig2 = sc_sb[h][:, 3:4]
                L = sc_sb[h][:, 4:5]
                step = sc_sb[h][:, 5:6]
                negstep = sc_sb[h][:, 6:7]
                beta = sc_sb[h][:, 7:8]
                nc.scalar.sqrt(sig2, nrm2)
                nc.vector.tensor_scalar(
                    L, sig2, 2.0 * P_SLACK * LSAFETY, LSAFETY,
                    op0=mybir.AluOpType.mult, op1=mybir.AluOpType.add,
                )
                nc.vector.reciprocal(step, L)
                nc.vector.tensor_scalar_mul(negstep, step, -1.0)
                # beta = (1-rs)/(1+rs) with rs = sqrt(step) = 1/sqrt(L)
                rs2 = sc_sb[h][:, 0:1]  # nrm2 dead now
                nc.scalar.sqrt(rs2, step)
                nc.vector.tensor_scalar(
                    num, rs2, -1.0, 1.0,
                    op0=mybir.AluOpType.mult, op1=mybir.AluOpType.add,
                )
                nc.vector.tensor_scalar(
                    den, rs2, 1.0, 1.0,
                    op0=mybir.AluOpType.mult, op1=mybir.AluOpType.add,
                )
                nc.vector.reciprocal(den, den)
                nc.vector.tensor_mul(beta, num, den)

            # ---- FISTA iterations (constant strongly-convex momentum) ----
            # Rotated software pipeline (uniform body): each iteration's y
            # transposes are injected into the NEXT PE chunk so every
            # transpose+scatter lands mid-chunk, with slack before its
            # consumer chunk.  Iteration k=0 re-runs k=-1's y1 transpose,
            # which is idempotent (sigma section scattered the same y).
            def emit_pointwise(h, x_old, x_new):
                negstep = sc_sb[h][:, 6:7]
                beta = sc_sb[h][:, 7:8]
                # g = y - x_raw ; u = 2p*w + g
                nc.vector.tensor_sub(g_sb[h][:], y_sb[h][:], xraw_sb[h][:])
                nc.vector.scalar_tensor_tensor(
                    u_sb[h][:], w_ps[h], 2.0 * P_SLACK, g_sb[h][:],
                    op0=mybir.AluOpType.mult, op1=mybir.AluOpType.add,
                )
                # x_new = clip(y - step*u)
                nc.vector.scalar_tensor_tensor(
                    x_new, u_sb[h][:], negstep, y_sb[h][:],
                    op0=mybir.AluOpType.mult, op1=mybir.AluOpType.add,
                )
                nc.vector.tensor_scalar(
                    x_new, x_new, 0.0, 100.0,
                    op0=mybir.AluOpType.max, op1=mybir.AluOpType.min,
                )
                # y = x_new + beta*(x_new - x_old)
                nc.vector.tensor_sub(g_sb[h][:], x_new, x_old)
                nc.vector.scalar_tensor_tensor(
                    y_sb[h][:], g_sb[h][:], beta,
                    x_new,
                    op0=mybir.AluOpType.mult, op1=mybir.AluOpType.add,
                )

            def emit_fista_iter(k):
                x_old = [xb, xa][k % 2]
                x_new = [xa, xb][k % 2]
                # z half0; inject prev iter's y1 transpose
                _emit_matvec_half(
                    nc, z_ps_t[0][:], y_diag, cur["AT"], M, 0,
                    inject_at=26,
                    inject_fn=lambda: transpose_scatter(
                        y_sb[1], y_diag, t1_ps_t[1], 1, N))
                nc.vector.tensor_sub(av_sb[0][:], z_ps[0], b_sb[0][:])
                nc.scalar.activation(r_sb[0][:], av_sb[0][:],
                                     mybir.ActivationFunctionType.Relu)
                # z half1; inject r0 transpose
                _emit_matvec_half(
                    nc, z_ps_t[1][:], y_diag, cur["AT"], M, 1,
                    inject_at=12,
                    inject_fn=lambda: transpose_scatter(
                        r_sb[0], r_diag, t2_ps_t[0], 0, M))
                nc.vector.tensor_sub(av_sb[1][:], z_ps[1], b_sb[1][:])
                nc.scalar.activation(r_sb[1][:], av_sb[1][:],
                                     mybir.ActivationFunctionType.Relu)
                # w half0; inject r1 transpose
                _emit_matvec_half(
                    nc, w_ps_t[0][:], r_diag, cur["A"], N, 0,
                    inject_at=12,
                    inject_fn=lambda: transpose_scatter(
                        r_sb[1], r_diag, t2_ps_t[1], 1, M))
                emit_pointwise(0, x_old[0], x_new[0])
                # w half1; inject y0 transpose (needs pw0)
                _emit_matvec_half(
                    nc, w_ps_t[1][:], r_diag, cur["A"], N, 1,
                    inject_at=34,
                    inject_fn=lambda: transpose_scatter(
                        y_sb[0], y_diag, t1_ps_t[0], 0, N))
                emit_pointwise(1, x_old[1], x_new[1])

            def emit_fista_group(t0):
                for k in range(UNROLL):
                    emit_fista_iter(k)

            if sim_unroll:
                for t0 in range(0, ITERS, UNROLL):
                    emit_fista_group(t0)
            else:
                with tc.For_i(0, ITERS, UNROLL, name="fista",
                              hint_engines=(mybir.EngineType.PE,)) as t0:
                    emit_fista_group(t0)

            # final x lives in the tile written by iteration ITERS-1 (k=3 -> xb)
            nc.sync.dma_start(out_d[ds(bj * (2 * BLK) + pair * BLK, BLK), :], xb_t[:])

        # block 0's A arrives before the loop; inside, each block prefetches
        # the next block's A into the buffer its predecessor just vacated
        emit_a_dma(0, 0)
        if sim_unroll:
            for bj in range(nblk // 2):
                emit_block(bj, 0, 0)
                emit_block(bj, 1, 1)
        else:
            with tc.For_i(0, NBLK // 2, 1, name="blk") as bj:
                emit_block(bj, 0, 0)
                emit_block(bj, 1, 1)

    if split_waits:
        _split_multiwait_insts(nc)
    return nc


_CACHED = {}


def _get_nc():
    if "nc" not in _CACHED:
        nc = bass.Bass("TRN2", target_bir_lowering=False, debug=False)
        build_kernel(nc)
        nc.finalize()
        _CACHED["nc"] = nc
    return _CACHED["nc"]


def _concat_in_maps(x_raw, A, b):
    per_core = []
    zA = np.zeros((1, M, BLK * N), ml_dtypes.bfloat16)
    zAT = np.zeros((1, N, BLK * M), ml_dtypes.bfloat16)
    for c in range(N_CORES):
        sl = slice(c * B_CORE, (c + 1) * B_CORE)
        Ac = A[sl].reshape(NBLK, BLK, M, N).astype(ml_dtypes.bfloat16)
        Ap = np.ascontiguousarray(Ac.transpose(0, 2, 1, 3)).reshape(NBLK, M, BLK * N)
        ATp = np.ascontiguousarray(Ac.transpose(0, 3, 1, 2)).reshape(NBLK, N, BLK * M)
        per_core.append({
            "x_raw": x_raw[sl],
            "Ap": np.concatenate([Ap, zA], axis=0),
            "ATp": np.concatenate([ATp, zAT], axis=0),
            "b": b[sl],
        })
    return per_core


def _build_trivial_nc():
    """Minimal DMA-roundtrip kernel used to calibrate the axon dispatch
    floor with the exact same timing path as the real kernel."""
    import concourse.tile as tile_mod
    nc = bass.Bass("TRN2", target_bir_lowering=False, debug=False)
    inp = nc.dram_tensor("tin", [128, 128], F32, kind="ExternalInput").ap()
    out = nc.dram_tensor("tout", [128, 128], F32, kind="ExternalOutput").ap()
    with tile_mod.TileContext(nc) as tc:
        with tc.tile_pool(name="p", bufs=1) as pool:
            t = pool.tile([128, 128], F32)
            nc.sync.dma_start(t[:], inp)
            nc.sync.dma_start(out, t[:])
    _split_multiwait_insts(nc)
    nc.finalize()
    return nc


def timed_runs_trivial(n=5):
    if "triv" not in _CACHED:
        _CACHED["triv"] = _build_trivial_nc()
    per_core = [{"tin": np.zeros((128, 128), np.float32)} for _ in range(N_CORES)]
    return _timed_exec(_CACHED["triv"], per_core, n)


def timed_runs(inputs, n=5):
    """Warm, device-resident-input executions; returns per-call wall ns."""
    x_raw = np.ascontiguousarray(inputs["x_raw"], np.float32)
    A = np.ascontiguousarray(inputs["A"], np.float32)
    b = np.ascontiguousarray(inputs["b"], np.float32)
    per_core = _concat_in_maps(x_raw, A, b)
    return _timed_exec(_get_nc(), per_core, n)


def timed_interleaved(inputs, n=24):
    """Alternate real-kernel and trivial-kernel timed calls so both sample
    the same axon-tunnel dispatch regime; returns (real_ns, trivial_ns)."""
    import time
    import jax

    x_raw = np.ascontiguousarray(inputs["x_raw"], np.float32)
    A = np.ascontiguousarray(inputs["A"], np.float32)
    b = np.ascontiguousarray(inputs["b"], np.float32)
    per_core = _concat_in_maps(x_raw, A, b)
    fn_r, args_r = _timed_prep(_get_nc(), per_core)
    if "triv" not in _CACHED:
        _CACHED["triv"] = _build_trivial_nc()
    triv_core = [{"tin": np.zeros((128, 128), np.float32)}
                 for _ in range(N_CORES)]
    fn_t, args_t = _timed_prep(_CACHED["triv"], triv_core)
    # warm both
    jax.block_until_ready(fn_r(*args_r))
    jax.block_until_ready(fn_t(*args_t))
    real, triv = [], []
    for _ in range(n):
        t0 = time.perf_counter()
        jax.block_until_ready(fn_t(*args_t))
        t1 = time.perf_counter()
        jax.block_until_ready(fn_r(*args_r))
        t2 = time.perf_counter()
        triv.append((t1 - t0) * 1e9)
        real.append((t2 - t1) * 1e9)
    return real, triv


def _timed_exec(nc, per_core, n):
    import time
    import jax

    fn, args = _timed_prep(nc, per_core)
    out = fn(*args)
    jax.block_until_ready(out)  # compile + warmup
    times = []
    for _ in range(n):
        t0 = time.perf_counter()
        out = fn(*args)
        jax.block_until_ready(out)
        times.append((time.perf_counter() - t0) * 1e9)
    return times


def _timed_prep(nc, per_core):
    import jax
    from jax.sharding import Mesh, PartitionSpec, NamedSharding
    from jax.experimental.shard_map import shard_map
    from concourse import bass2jax

    bass2jax.install_neuronx_cc_hook()

    in_names, out_names, out_avals = [], [], []
    for alloc in nc.m.functions[0].allocations:
        if not isinstance(alloc, mybir.MemoryLocationSet):
            continue
        name = alloc.memorylocations[0].name
        if alloc.kind == "ExternalInput":
            in_names.append(name)
        elif alloc.kind == "ExternalOutput":
            out_names.append(name)
            out_avals.append(jax.core.ShapedArray(
                tuple(alloc.tensor_shape), mybir.dt.np(alloc.dtype)))
    pid_name = nc.partition_id_tensor.name if nc.partition_id_tensor else None
    if pid_name is not None and pid_name in in_names:
        in_names.remove(pid_name)

    all_names = in_names + out_names
    if pid_name is not None:
        all_names = all_names + [pid_name]

    def _body(*args):
        operands = list(args)
        if pid_name is not None:
            operands.append(bass2jax.partition_id_tensor())
        outs = bass2jax._bass_exec_p.bind(
            *operands,
            out_avals=tuple(out_avals),
            in_names=tuple(all_names),
            out_names=tuple(out_names),
            lowering_input_output_aliases=(),
            sim_require_finite=True,
            sim_require_nnan=True,
            nc=nc,
        )
        return tuple(outs)

    devices = jax.devices()[:N_CORES]
    mesh = Mesh(np.asarray(devices), ("core",))
    nin = len(in_names) + len(out_names)
    fn = jax.jit(
        shard_map(_body, mesh=mesh, in_specs=(PartitionSpec("core"),) * nin,
                  out_specs=(PartitionSpec("core"),) * len(out_names),
                  check_rep=False),
        keep_unused=True,
    )
    sh = NamedSharding(mesh, PartitionSpec("core"))
    concat = [np.concatenate([pc[nm] for pc in per_core], axis=0) for nm in in_names]
    zeros = [np.zeros((N_CORES * av.shape[0], *av.shape[1:]), av.dtype)
             for av in out_avals]
    args = [jax.device_put(v, sh) for v in concat + zeros]
    return fn, args


def kernel(x_raw, A, b, lower, upper):
    x_raw = np.ascontiguousarray(x_raw, np.float32)
    A = np.ascontiguousarray(A, np.float32)
    b = np.ascontiguousarray(b, np.float32)

    nc = _get_nc()
    in_maps = _concat_in_maps(x_raw, A, b)
    res = run_bass_kernel_spmd(nc, in_maps, core_ids=list(range(N_CORES)))
    out = np.concatenate([res.results[c]["x_out"] for c in range(N_CORES)], axis=0)
    return out.astype(np.float32)
